# revision 14
# baseline (speedup 1.0000x reference)
"""Trainium2 Bass kernel for nn_AttentionHead (single-head attention with
pre-softmax tril zeroing). B=8, S=2048, E=1024, H=64.

Sharding: data-parallel over batch - one batch element per NeuronCore,
no collectives. Each core computes, for its batch b:

  q = y@Wq + bq ; k' = x@(Wk/8) + (bk/8) ; v = x@Wv + bv
  scores[r, j] = q[r].k'[j] for j<=r, 0 for j>r      (tril PRE-softmax)
  attn = softmax(scores, -1)  -> masked entries contribute exp(0)=1
  out = attn @ v

v4 design:
  - x,y host-cast to bf16 and host-pre-transposed to [E, S]; plain
    contiguous DMA loads on both HWDGE queues (x on sync, y on scalar)
  - two fp32 dummy matmuls at t=0 warm the PE HAM clock gate
  - QKV is sc4-outer/e-inner so q-chunk c + its k/v blocks evacuate as
    soon as their accumulations stop; attention column c overlaps the
    remaining QKV chunks
  - scores computed transposed (st[k, q]) in row-packed PAIRS: two K=64
    matmuls run concurrently in the PE array via tile_position (0,0) /
    (64,0), with kT/qT duplicated on partitions 64-127
  - diagonal-band blocks: exp only the non-fully-masked column range;
    post-exp affine_select on GpSimd fills masked cells with exp(0)=1
  - never-materialized upper blocks are closed-form: pv += suffix-sum(v),
    Z += count (ones column fused into v_aug via xbar DMA transpose)
  - softmax denominator via the ones row (row 64) of the PV accumulator;
    no max-subtraction (scores ~ N(0,1), f32 exp safe)
"""

import numpy as np

import concourse.bass as bass
import concourse.mybir as mybir
from concourse.tile import TileContext

S, E, H = 2048, 1024, 64
SC = S // 128   # 16 s-chunks (key blocks)
ECH = E // 128  # 8 e-chunks
NQ = 4          # q-chunks of 512
F32 = mybir.dt.float32
BF16 = mybir.dt.bfloat16
AF = mybir.ActivationFunctionType

_SPLIT_COUNTER = [0]


def _split_multi_waits(nc, ev_cap=1):
    """This container's walrus build accepts at most 1 sem-wait per
    instruction (2 on EventSemaphore); move excess waits onto EvSem
    instructions inserted just before, on the same engine."""
    for f in nc.m.functions:
        for bb in f.blocks:
            ins_list = bb.instructions
            need = False
            for ins in ins_list:
                si = ins.sync_info
                if si is None:
                    continue
                cap = 2 if isinstance(ins, mybir.InstEventSemaphore) else 1
                if len(si.on_wait) > cap:
                    need = True
                    break
            if not need:
                continue
            new_list = []
            for ins in ins_list:
                si = ins.sync_info
                cap = 2 if isinstance(ins, mybir.InstEventSemaphore) else 1
                if si is not None and len(si.on_wait) > cap:
                    waits = list(si.on_wait)
                    keep = waits[-cap:]
                    head = waits[:-cap]
                    for i in range(0, len(head), ev_cap):
                        _SPLIT_COUNTER[0] += 1
                        ev = mybir.InstEventSemaphore(
                            name=f"EVSPLIT-{_SPLIT_COUNTER[0]}",
                            engine=ins.engine,
                            ins=[],
                            outs=[],
                            sync_info=mybir.SyncInfo(
                                on_wait=head[i:i + ev_cap], on_update=[]
                            ),
                        )
                        nc.register_instruction(ev)
                        new_list.append(ev)
                    ins.sync_info = mybir.SyncInfo(
                        on_wait=keep, on_update=list(si.on_update)
                    )
                new_list.append(ins)
            bb.instructions = new_list


def _build():
    nc = bass.Bass()
    # x, y pre-cast bf16 AND pre-transposed to [E, S] on host
    x_ext = nc.declare_dram_parameter("x", [E, S], BF16, isOutput=False)
    y_ext = nc.declare_dram_parameter("y", [E, S], BF16, isOutput=False)
    # weights host-packed: wkv [128, ECH*128] ([Wk' | Wv] per e-chunk),
    # wq [128, ECH*64]
    wkv_ext = nc.declare_dram_parameter("wkv", [128, ECH * 128], BF16,
                                        isOutput=False)
    wq_ext = nc.declare_dram_parameter("wq", [128, ECH * H], BF16,
                                       isOutput=False)
    bq_ext = nc.declare_dram_parameter("bq", [H, 1], F32, isOutput=False)
    bk_ext = nc.declare_dram_parameter("bk", [H, 1], F32, isOutput=False)
    bv_ext = nc.declare_dram_parameter("bv", [H, 1], F32, isOutput=False)
    out_ext = nc.declare_dram_parameter("out", [S, H], F32, isOutput=True)

    with TileContext(nc) as tc:
        with (
            tc.tile_pool(name="consts", bufs=1) as consts,
            tc.tile_pool(name="bigT", bufs=1) as bigT,
            tc.tile_pool(name="expp", bufs=3) as expp,
            tc.tile_pool(name="outp", bufs=2) as outp,
        ):
            # ---- constants ----
            ident_f = consts.tile([128, 128], F32)
            nc.vector.memset(ident_f, 1.0)
            nc.gpsimd.affine_select(
                out=ident_f, in_=ident_f,
                pattern=[[-1, 128]], channel_multiplier=1, base=0,
                compare_op=mybir.AluOpType.is_equal, fill=0.0,
            )

            # ---- PE warm-up: two fp32 matmuls (~3.4us) flip the HAM ----
            scr = consts.tile([128, 512], F32, tag="scr")
            nc.vector.memset(scr, 0.0)
            with tc.tile_pool(name="psW", bufs=1, space="PSUM") as psW:
                wm = psW.tile([128, 512], F32, tag="warm")
                nc.tensor.matmul(wm, lhsT=ident_f, rhs=scr,
                                 start=True, stop=True)
                nc.tensor.matmul(wm, lhsT=ident_f, rhs=scr,
                                 start=True, stop=True)

            # ---- weights & biases ----
            w_kv = consts.tile([128, ECH * 128], BF16, tag="w_kv")
            w_q = consts.tile([128, ECH * H], BF16, tag="w_q")
            nc.sync.dma_start(out=w_kv, in_=wkv_ext[:, :])
            nc.scalar.dma_start(out=w_q, in_=wq_ext[:, :])
            bias_sb = {}
            for name, bext in (("q", bq_ext), ("k", bk_ext), ("v", bv_ext)):
                bs = consts.tile([H, 1], F32, tag=f"b_{name}",
                                 name=f"bias_{name}")
                nc.sync.dma_start(out=bs, in_=bext[:, :])
                bias_sb[name] = bs

            # ---- phase A: load pre-transposed x, y; dual HWDGE queues ----
            xT = bigT.tile([128, ECH * S], BF16, tag="xT")
            yT = bigT.tile([128, ECH * S], BF16, tag="yT")
            for e in range(ECH):
                nc.sync.dma_start(
                    out=xT[:, e * S:(e + 1) * S],
                    in_=x_ext[e * 128:(e + 1) * 128, :],
                )
                nc.scalar.dma_start(
                    out=yT[:, e * S:(e + 1) * S],
                    in_=y_ext[e * 128:(e + 1) * 128, :],
                )

            # kT/qT duplicated on partitions 64:128 for row-packed scores
            qTd = bigT.tile([128, S], BF16, tag="qTd")
            kTd = bigT.tile([128, S], BF16, tag="kTd")
            vT = bigT.tile([H, S], BF16, tag="vT")
            v_aug = bigT.tile([128, SC * (H + 1)], BF16, tag="vaug")
            nc.vector.memset(v_aug, 1.0)
            v_nat = bigT.tile([128, SC * H], BF16, tag="vnat")
            vsuf = []
            wins = [None]
            for c in range(NQ):
                va = consts.tile([H + 1, 1], F32, tag=f"vsuf{c}",
                                 name=f"vsuf_{c}")
                nc.vector.memset(va, 0.0)
                if c < NQ - 1:
                    nc.vector.memset(va[H:H + 1, :],
                                     float((NQ - 1 - c) * 512))
                vsuf.append(va)
            for g in (1, 2):
                wins.append(consts.tile([H, 1], F32, tag=f"win{g}",
                                        name=f"win_{g}"))

            # ---- phase B: QKV projections, sc4-outer so attention can
            # start per q-chunk; phase E interleaved below ----
            with (
                tc.tile_pool(name="psQ", bufs=1, space="PSUM") as psQ,
            ):
                kv_accs = [
                    psQ.tile([128, 512], F32, tag="kvacc", bufs=NQ,
                             name=f"kvacc_{i}")
                    for i in range(NQ)
                ]
                q_accs = [
                    psQ.tile([H, 512], F32, tag="qacc", bufs=NQ,
                             name=f"qacc_{i}")
                    for i in range(NQ)
                ]
                for sc4 in range(NQ):
                    sl = slice(sc4 * 512, (sc4 + 1) * 512)
                    for e in range(ECH):
                        nc.tensor.matmul(
                            kv_accs[sc4],
                            lhsT=w_kv[:, e * 128:(e + 1) * 128],
                            rhs=xT[:, e * S + sc4 * 512: e * S + (sc4 + 1) * 512],
                            start=(e == 0),
                            stop=(e == ECH - 1),
                        )
                        nc.tensor.matmul(
                            q_accs[sc4],
                            lhsT=w_q[:, e * H:(e + 1) * H],
                            rhs=yT[:, e * S + sc4 * 512: e * S + (sc4 + 1) * 512],
                            start=(e == 0),
                            stop=(e == ECH - 1),
                        )
                    # evacuate: kT/qT lo (DVE), vT (ACT); hi duplicates for
                    # row-packed scores go cross-partition via SBUF-SBUF DMA
                    nc.vector.tensor_scalar_add(
                        out=kTd[0:H, sl], in0=kv_accs[sc4][0:H, :],
                        scalar1=bias_sb["k"])
                    nc.vector.tensor_scalar_add(
                        out=qTd[0:H, sl], in0=q_accs[sc4],
                        scalar1=bias_sb["q"])
                    nc.sync.dma_start(out=kTd[H:128, sl], in_=kTd[0:H, sl])
                    nc.sync.dma_start(out=qTd[H:128, sl], in_=qTd[0:H, sl])
                    nc.scalar.activation(
                        out=vT[:, sl], in_=kv_accs[sc4][H:128, :],
                        func=AF.Identity, bias=bias_sb["v"])
                    # v natural for this sc4 (4 key blocks) + suffix window
                    nc.sync.dma_start(
                        out=v_nat.rearrange(
                            "p (j h) -> p j h", h=H
                        )[:, 4 * sc4:4 * sc4 + 4, :],
                        in_=vT[:, sl], transpose=True,
                    )
                    nc.vector.tensor_copy(
                        v_aug.rearrange(
                            "p (j h) -> p j h", h=H + 1
                        )[:, 4 * sc4:4 * sc4 + 4, 0:H],
                        v_nat.rearrange(
                            "p (j h) -> p j h", h=H
                        )[:, 4 * sc4:4 * sc4 + 4, :],
                    )
                    # suffix-sum windows: window g = vT cols [512g, 512g+512)
                    if sc4 in (1, 2):
                        nc.vector.reduce_sum(
                            out=wins[sc4], in_=vT[:, sl],
                            axis=mybir.AxisListType.X)
                    elif sc4 == 3:
                        nc.vector.reduce_sum(
                            out=vsuf[2][0:H, :], in_=vT[:, sl],
                            axis=mybir.AxisListType.X)
                        nc.vector.tensor_add(
                            out=vsuf[1][0:H, :], in0=vsuf[2][0:H, :],
                            in1=wins[2])
                        nc.vector.tensor_add(
                            out=vsuf[0][0:H, :], in0=vsuf[1][0:H, :],
                            in1=wins[1])

            # ---- phase E: attention, column-major over q-chunks ----
            with tc.tile_pool(name="psE", bufs=1, space="PSUM") as psE:
                for c in range(NQ):
                    pv = psE.tile([H + 1, 512], F32, tag="pv", bufs=2,
                                  name=f"pv_{c}")
                    nb = 4 * c + 4
                    exs = []
                    for b2 in range(nb // 2):
                        for half in range(2):
                            b = 2 * b2 + half
                            st = psE.tile([128, 512], F32, tag="st", bufs=4)
                            lo = half * H
                            nc.tensor.matmul(
                                st,
                                lhsT=kTd[lo:lo + H, b * 128:(b + 1) * 128],
                                rhs=qTd[lo:lo + H, c * 512:(c + 1) * 512],
                                start=True,
                                stop=True,
                                tile_position=(lo, 0),
                            )
                            ex = expp.tile([128, 512], BF16, tag="expst",
                                           bufs=16)
                            if b // 4 == c:
                                d = b - 4 * c
                                nc.scalar.activation(
                                    out=ex[:, 128 * d:], in_=st[:, 128 * d:],
                                    func=AF.Exp)
                                w = 128 * (d + 1)
                                nc.gpsimd.affine_select(
                                    out=ex[:, 0:w], in_=ex[:, 0:w],
                                    pattern=[[1, w]], channel_multiplier=-1,
                                    base=-128 * d,
                                    compare_op=mybir.AluOpType.is_ge,
                                    fill=1.0,
                                )
                            else:
                                nc.scalar.activation(out=ex, in_=st,
                                                     func=AF.Exp)
                            exs.append(ex)
                        for half in range(2):
                            b = 2 * b2 + half
                            nc.tensor.matmul(
                                pv,
                                lhsT=v_aug[:, b * (H + 1):(b + 1) * (H + 1)],
                                rhs=exs[b],
                                start=(b == 0),
                                stop=(b == nb - 1),
                            )
                    # finish: closed-form upper + normalize + store
                    # (copy first so pv's bank frees without waiting vsuf)
                    r0 = c * 512
                    sbn = outp.tile([H + 1, 512], F32, tag="sbn")
                    nc.vector.tensor_copy(sbn, pv)
                    nc.vector.tensor_scalar_add(out=sbn, in0=sbn,
                                                scalar1=vsuf[c])
                    for j4 in range(4):
                        pt = psE.tile([128, H + 1], F32, tag="pt", bufs=2)
                        nc.tensor.transpose(
                            pt, sbn[:, j4 * 128:(j4 + 1) * 128],
                            ident_f[0:H + 1, 0:H + 1],
                        )
                        rcp = outp.tile([128, 1], F32, tag="rcp")
                        nc.vector.reciprocal(rcp, pt[:, H:H + 1])
                        of = outp.tile([128, H], F32, tag="of")
                        nc.vector.tensor_scalar_mul(out=of, in0=pt[:, 0:H],
                                                    scalar1=rcp)
                        r = r0 + j4 * 128
                        nc.sync.dma_start(out=out_ext[r:r + 128, :], in_=of)

    _split_multi_waits(nc)
    return nc


LAST_EXEC_TIME_NS = None
_CACHE = {}


def kernel(x, y, Wq, bq, Wk, bk, Wv, bv):
    """Full-input entry point: shards batch over 8 NeuronCores (one batch
    element per core), runs the Bass kernel, gathers the full output."""
    global LAST_EXEC_TIME_NS
    import os

    import ml_dtypes
    from concourse.bass_utils import run_bass_kernel_spmd

    if "nc" not in _CACHE:
        _CACHE["nc"] = _build()
    nc = _CACHE["nc"]

    bf = ml_dtypes.bfloat16
    x = np.asarray(x, np.float32)
    y = np.asarray(y, np.float32)

    # host-side weight packing: [128, ECH, 128] -> [128, ECH*128]
    wk8 = (np.asarray(Wk, np.float32) * 0.125).astype(bf).reshape(ECH, 128, H)
    wv2 = np.asarray(Wv, np.float32).astype(bf).reshape(ECH, 128, H)
    wkv = np.ascontiguousarray(
        np.concatenate([wk8, wv2], axis=2).transpose(1, 0, 2)
    ).reshape(128, ECH * 128)
    wq2 = np.ascontiguousarray(
        np.asarray(Wq, np.float32).astype(bf).reshape(ECH, 128, H)
        .transpose(1, 0, 2)
    ).reshape(128, ECH * H)
    bqc = np.ascontiguousarray(np.asarray(bq, np.float32).reshape(H, 1))
    bkc = np.ascontiguousarray(
        np.asarray(bk, np.float32).reshape(H, 1) * 0.125)
    bvc = np.ascontiguousarray(np.asarray(bv, np.float32).reshape(H, 1))

    in_maps = []
    for b in range(8):
        xe = np.ascontiguousarray(x[b].astype(bf).T)
        ye = np.ascontiguousarray(y[b].astype(bf).T)
        in_maps.append({
            "x": xe, "y": ye,
            "wkv": wkv, "wq": wq2,
            "bq": bqc, "bk": bkc, "bv": bvc,
        })

    trace = bool(os.environ.get("ATTN_TRACE"))
    res = run_bass_kernel_spmd(nc, in_maps, core_ids=list(range(8)),
                               trace=trace)
    if trace:
        LAST_EXEC_TIME_NS = res.exec_time_ns
    return np.stack([res.results[i]["out"] for i in range(8)]).astype(
        np.float32)


# revision 17
# speedup vs baseline: 1.2315x; 1.2315x over previous
"""Trainium2 Bass kernel for nn_AttentionHead (single-head attention with
pre-softmax tril zeroing). B=8, S=2048, E=1024, H=64.

Sharding: data-parallel over batch - one batch element per NeuronCore,
no collectives. Each core computes, for its batch b:

  q = y@Wq + bq ; k' = x@(Wk/8) + (bk/8) ; v = x@Wv + bv
  scores[r, j] = q[r].k'[j] for j<=r, 0 for j>r      (tril PRE-softmax)
  attn = softmax(scores, -1)  -> masked entries contribute exp(0)=1
  out = attn @ v

v4 design:
  - x,y host-cast to bf16 and host-pre-transposed to [E, S]; plain
    contiguous DMA loads on both HWDGE queues (x on sync, y on scalar)
  - two fp32 dummy matmuls at t=0 warm the PE HAM clock gate
  - QKV is sc4-outer/e-inner so q-chunk c + its k/v blocks evacuate as
    soon as their accumulations stop; attention column c overlaps the
    remaining QKV chunks
  - scores computed transposed (st[k, q]) in row-packed PAIRS: two K=64
    matmuls run concurrently in the PE array via tile_position (0,0) /
    (64,0), with kT/qT duplicated on partitions 64-127
  - diagonal-band blocks: exp only the non-fully-masked column range;
    post-exp affine_select on GpSimd fills masked cells with exp(0)=1
  - never-materialized upper blocks are closed-form: pv += suffix-sum(v),
    Z += count (ones column fused into v_aug via xbar DMA transpose)
  - softmax denominator via the ones row (row 64) of the PV accumulator;
    no max-subtraction (scores ~ N(0,1), f32 exp safe)
"""

import numpy as np

import concourse.bass as bass
import concourse.mybir as mybir
from concourse.tile import TileContext

S, E, H = 2048, 1024, 64
SC = S // 128   # 16 s-chunks (key blocks)
ECH = E // 128  # 8 e-chunks
NQ = 4          # q-chunks of 512
F32 = mybir.dt.float32
BF16 = mybir.dt.bfloat16
AF = mybir.ActivationFunctionType

_SPLIT_COUNTER = [0]


def _split_multi_waits(nc, ev_cap=1):
    """This container's walrus build accepts at most 1 sem-wait per
    instruction (2 on EventSemaphore); move excess waits onto EvSem
    instructions inserted just before, on the same engine."""
    for f in nc.m.functions:
        for bb in f.blocks:
            ins_list = bb.instructions
            need = False
            for ins in ins_list:
                si = ins.sync_info
                if si is None:
                    continue
                cap = 2 if isinstance(ins, mybir.InstEventSemaphore) else 1
                if len(si.on_wait) > cap:
                    need = True
                    break
            if not need:
                continue
            new_list = []
            for ins in ins_list:
                si = ins.sync_info
                cap = 2 if isinstance(ins, mybir.InstEventSemaphore) else 1
                if si is not None and len(si.on_wait) > cap:
                    waits = list(si.on_wait)
                    keep = waits[-cap:]
                    head = waits[:-cap]
                    for i in range(0, len(head), ev_cap):
                        _SPLIT_COUNTER[0] += 1
                        ev = mybir.InstEventSemaphore(
                            name=f"EVSPLIT-{_SPLIT_COUNTER[0]}",
                            engine=ins.engine,
                            ins=[],
                            outs=[],
                            sync_info=mybir.SyncInfo(
                                on_wait=head[i:i + ev_cap], on_update=[]
                            ),
                        )
                        nc.register_instruction(ev)
                        new_list.append(ev)
                    ins.sync_info = mybir.SyncInfo(
                        on_wait=keep, on_update=list(si.on_update)
                    )
                new_list.append(ins)
            bb.instructions = new_list


def _build():
    nc = bass.Bass()
    # x, y pre-cast bf16 AND pre-transposed to [E, S] on host
    x_ext = nc.declare_dram_parameter("x", [E, S], BF16, isOutput=False)
    y_ext = nc.declare_dram_parameter("y", [E, S], BF16, isOutput=False)
    # weights host-packed: wkv [128, ECH*128] ([Wk' | Wv] per e-chunk),
    # wq [128, ECH*64]
    wkv_ext = nc.declare_dram_parameter("wkv", [128, ECH * 128], BF16,
                                        isOutput=False)
    wq_ext = nc.declare_dram_parameter("wq", [128, ECH * H], BF16,
                                       isOutput=False)
    bq_ext = nc.declare_dram_parameter("bq", [H, 1], F32, isOutput=False)
    bk_ext = nc.declare_dram_parameter("bk", [H, 1], F32, isOutput=False)
    bv_ext = nc.declare_dram_parameter("bv", [H, 1], F32, isOutput=False)
    out_ext = nc.declare_dram_parameter("out", [S, H], F32, isOutput=True)

    with TileContext(nc) as tc:
        with (
            tc.tile_pool(name="consts", bufs=1) as consts,
            tc.tile_pool(name="bigT", bufs=1) as bigT,
            tc.tile_pool(name="expp", bufs=3) as expp,
            tc.tile_pool(name="outp", bufs=2) as outp,
        ):
            # ---- constants ----
            ident_f = consts.tile([128, 128], F32)
            nc.vector.memset(ident_f, 1.0)
            nc.gpsimd.affine_select(
                out=ident_f, in_=ident_f,
                pattern=[[-1, 128]], channel_multiplier=1, base=0,
                compare_op=mybir.AluOpType.is_equal, fill=0.0,
            )
            ident_bf = consts.tile([128, 128], BF16)
            nc.vector.memset(ident_bf, 1.0)
            nc.gpsimd.affine_select(
                out=ident_bf, in_=ident_bf,
                pattern=[[-1, 128]], channel_multiplier=1, base=0,
                compare_op=mybir.AluOpType.is_equal, fill=0.0,
            )

            # ---- PE warm-up: two fp32 matmuls (~3.4us) flip the HAM ----
            scr = consts.tile([128, 512], F32, tag="scr")
            nc.vector.memset(scr, 0.0)
            with tc.tile_pool(name="psW", bufs=1, space="PSUM") as psW:
                wm = psW.tile([128, 512], F32, tag="warm")
                nc.tensor.matmul(wm, lhsT=ident_f, rhs=scr,
                                 start=True, stop=True)
                nc.tensor.matmul(wm, lhsT=ident_f, rhs=scr,
                                 start=True, stop=True)

            # ---- weights & biases ----
            w_kv = consts.tile([128, ECH * 128], BF16, tag="w_kv")
            w_q = consts.tile([128, ECH * H], BF16, tag="w_q")
            nc.sync.dma_start(out=w_kv, in_=wkv_ext[:, :])
            nc.scalar.dma_start(out=w_q, in_=wq_ext[:, :])
            bias_sb = {}
            for name, bext in (("q", bq_ext), ("k", bk_ext), ("v", bv_ext)):
                bs = consts.tile([H, 1], F32, tag=f"b_{name}",
                                 name=f"bias_{name}")
                nc.sync.dma_start(out=bs, in_=bext[:, :])
                bias_sb[name] = bs

            # ---- phase A: load pre-transposed x, y; dual HWDGE queues ----
            xT = bigT.tile([128, ECH * S], BF16, tag="xT")
            yT = bigT.tile([128, ECH * S], BF16, tag="yT")
            for e in range(ECH):
                nc.sync.dma_start(
                    out=xT[:, e * S:(e + 1) * S],
                    in_=x_ext[e * 128:(e + 1) * 128, :],
                )
                nc.scalar.dma_start(
                    out=yT[:, e * S:(e + 1) * S],
                    in_=y_ext[e * 128:(e + 1) * 128, :],
                )

            # kT/qT duplicated on partitions 64:128 for row-packed scores
            qTd = bigT.tile([128, S], BF16, tag="qTd")
            kTd = bigT.tile([128, S], BF16, tag="kTd")
            vT = bigT.tile([H, S], BF16, tag="vT")
            v_aug = bigT.tile([128, SC * (H + 1)], BF16, tag="vaug")
            nc.vector.memset(v_aug, 1.0)
            v_nat = bigT.tile([128, SC * H], BF16, tag="vnat")
            vsuf = []
            wins = [None]
            for c in range(NQ):
                va = consts.tile([H + 1, 1], F32, tag=f"vsuf{c}",
                                 name=f"vsuf_{c}")
                nc.vector.memset(va, 0.0)
                if c < NQ - 1:
                    nc.vector.memset(va[H:H + 1, :],
                                     float((NQ - 1 - c) * 512))
                vsuf.append(va)
            for g in (1, 2):
                wins.append(consts.tile([H, 1], F32, tag=f"win{g}",
                                        name=f"win_{g}"))

            # ---- phase B: QKV projections in two sc4-groups; within a
            # group, 4 PSUM accumulators rotate so accumulate-drain
            # hazards never serialize consecutive matmuls ----
            with (
                tc.tile_pool(name="psQ", bufs=1, space="PSUM") as psQ,
            ):
                for ga, gb in ((0, 1), (2, 3)):
                    kv_accs = {
                        s: psQ.tile([128, 512], F32, tag="kvacc", bufs=2,
                                    name=f"kvacc_{s}")
                        for s in (ga, gb)
                    }
                    q_accs = {
                        s: psQ.tile([H, 512], F32, tag="qacc", bufs=2,
                                    name=f"qacc_{s}")
                        for s in (ga, gb)
                    }
                    for e in range(ECH):
                        for sc4 in (ga, gb):
                            nc.tensor.matmul(
                                kv_accs[sc4],
                                lhsT=w_kv[:, e * 128:(e + 1) * 128],
                                rhs=xT[:, e * S + sc4 * 512:
                                       e * S + (sc4 + 1) * 512],
                                start=(e == 0),
                                stop=(e == ECH - 1),
                            )
                            nc.tensor.matmul(
                                q_accs[sc4],
                                lhsT=w_q[:, e * H:(e + 1) * H],
                                rhs=yT[:, e * S + sc4 * 512:
                                       e * S + (sc4 + 1) * 512],
                                start=(e == 0),
                                stop=(e == ECH - 1),
                            )
                    for sc4 in (ga, gb):
                        sl = slice(sc4 * 512, (sc4 + 1) * 512)
                        # kT/vT on ACT, qT on DVE (parallel evac streams);
                        # hi duplicates for row-packed scores cross
                        # partitions via SBUF-SBUF DMA
                        nc.scalar.activation(
                            out=kTd[0:H, sl], in_=kv_accs[sc4][0:H, :],
                            func=AF.Identity, bias=bias_sb["k"])
                        nc.vector.tensor_scalar_add(
                            out=qTd[0:H, sl], in0=q_accs[sc4],
                            scalar1=bias_sb["q"])
                        nc.sync.dma_start(out=kTd[H:128, sl],
                                          in_=kTd[0:H, sl])
                        nc.sync.dma_start(out=qTd[H:128, sl],
                                          in_=qTd[0:H, sl])
                        nc.scalar.activation(
                            out=vT[:, sl], in_=kv_accs[sc4][H:128, :],
                            func=AF.Identity, bias=bias_sb["v"])
                        # v natural for this sc4 (4 key blocks)
                        nc.sync.dma_start(
                            out=v_nat.rearrange(
                                "p (j h) -> p j h", h=H
                            )[:, 4 * sc4:4 * sc4 + 4, :],
                            in_=vT[:, sl], transpose=True,
                        )
                        nc.vector.tensor_copy(
                            v_aug.rearrange(
                                "p (j h) -> p j h", h=H + 1
                            )[:, 4 * sc4:4 * sc4 + 4, 0:H],
                            v_nat.rearrange(
                                "p (j h) -> p j h", h=H
                            )[:, 4 * sc4:4 * sc4 + 4, :],
                        )
                        # suffix-sum windows (cols [512g, 512g+512))
                        if sc4 in (1, 2):
                            nc.vector.reduce_sum(
                                out=wins[sc4], in_=vT[:, sl],
                                axis=mybir.AxisListType.X)
                        elif sc4 == 3:
                            nc.vector.reduce_sum(
                                out=vsuf[2][0:H, :], in_=vT[:, sl],
                                axis=mybir.AxisListType.X)
                            nc.vector.tensor_add(
                                out=vsuf[1][0:H, :], in0=vsuf[2][0:H, :],
                                in1=wins[2])
                            nc.vector.tensor_add(
                                out=vsuf[0][0:H, :], in0=vsuf[1][0:H, :],
                                in1=wins[1])

            # ---- phase E: attention, column-major over q-chunks ----
            with tc.tile_pool(name="psE", bufs=1, space="PSUM") as psE:
                for c in range(NQ):
                    pv = psE.tile([H + 1, 512], F32, tag="pv", bufs=2,
                                  name=f"pv_{c}")
                    nb = 4 * c + 4
                    exs = []
                    for b2 in range(nb // 2):
                        for half in range(2):
                            b = 2 * b2 + half
                            st = psE.tile([128, 512], F32, tag="st", bufs=4)
                            lo = half * H
                            nc.tensor.matmul(
                                st,
                                lhsT=kTd[lo:lo + H, b * 128:(b + 1) * 128],
                                rhs=qTd[lo:lo + H, c * 512:(c + 1) * 512],
                                start=True,
                                stop=True,
                                tile_position=(lo, 0),
                            )
                            ex = expp.tile([128, 512], BF16, tag="expst",
                                           bufs=16)
                            if b // 4 == c:
                                d = b - 4 * c
                                nc.scalar.activation(
                                    out=ex[:, 128 * d:], in_=st[:, 128 * d:],
                                    func=AF.Exp)
                                w = 128 * (d + 1)
                                nc.gpsimd.affine_select(
                                    out=ex[:, 0:w], in_=ex[:, 0:w],
                                    pattern=[[1, w]], channel_multiplier=-1,
                                    base=-128 * d,
                                    compare_op=mybir.AluOpType.is_ge,
                                    fill=1.0,
                                )
                            else:
                                nc.scalar.activation(out=ex, in_=st,
                                                     func=AF.Exp)
                            exs.append(ex)
                        for half in range(2):
                            b = 2 * b2 + half
                            nc.tensor.matmul(
                                pv,
                                lhsT=v_aug[:, b * (H + 1):(b + 1) * (H + 1)],
                                rhs=exs[b],
                                start=(b == 0),
                                stop=(b == nb - 1),
                            )
                    # finish: closed-form upper + normalize + store
                    # (copy first so pv's bank frees without waiting vsuf;
                    # bf16 transpose is 1cy/row vs fp32's dual-pass)
                    r0 = c * 512
                    sbn = outp.tile([H + 1, 512], BF16, tag="sbn")
                    nc.vector.tensor_copy(sbn, pv)
                    nc.vector.tensor_scalar_add(out=sbn, in0=sbn,
                                                scalar1=vsuf[c])
                    for j4 in range(4):
                        pt = psE.tile([128, H + 1], BF16, tag="pt", bufs=2)
                        nc.tensor.transpose(
                            pt, sbn[:, j4 * 128:(j4 + 1) * 128],
                            ident_bf[0:H + 1, 0:H + 1],
                        )
                        rcp = outp.tile([128, 1], F32, tag="rcp")
                        nc.vector.reciprocal(rcp, pt[:, H:H + 1])
                        of = outp.tile([128, H], F32, tag="of")
                        nc.vector.tensor_scalar_mul(out=of, in0=pt[:, 0:H],
                                                    scalar1=rcp)
                        r = r0 + j4 * 128
                        nc.sync.dma_start(out=out_ext[r:r + 128, :], in_=of)

    _split_multi_waits(nc)
    return nc


LAST_EXEC_TIME_NS = None
_CACHE = {}


def kernel(x, y, Wq, bq, Wk, bk, Wv, bv):
    """Full-input entry point: shards batch over 8 NeuronCores (one batch
    element per core), runs the Bass kernel, gathers the full output."""
    global LAST_EXEC_TIME_NS
    import os

    import ml_dtypes
    from concourse.bass_utils import run_bass_kernel_spmd

    if "nc" not in _CACHE:
        _CACHE["nc"] = _build()
    nc = _CACHE["nc"]

    bf = ml_dtypes.bfloat16
    x = np.asarray(x, np.float32)
    y = np.asarray(y, np.float32)

    # host-side weight packing: [128, ECH, 128] -> [128, ECH*128]
    wk8 = (np.asarray(Wk, np.float32) * 0.125).astype(bf).reshape(ECH, 128, H)
    wv2 = np.asarray(Wv, np.float32).astype(bf).reshape(ECH, 128, H)
    wkv = np.ascontiguousarray(
        np.concatenate([wk8, wv2], axis=2).transpose(1, 0, 2)
    ).reshape(128, ECH * 128)
    wq2 = np.ascontiguousarray(
        np.asarray(Wq, np.float32).astype(bf).reshape(ECH, 128, H)
        .transpose(1, 0, 2)
    ).reshape(128, ECH * H)
    bqc = np.ascontiguousarray(np.asarray(bq, np.float32).reshape(H, 1))
    bkc = np.ascontiguousarray(
        np.asarray(bk, np.float32).reshape(H, 1) * 0.125)
    bvc = np.ascontiguousarray(np.asarray(bv, np.float32).reshape(H, 1))

    in_maps = []
    for b in range(8):
        xe = np.ascontiguousarray(x[b].astype(bf).T)
        ye = np.ascontiguousarray(y[b].astype(bf).T)
        in_maps.append({
            "x": xe, "y": ye,
            "wkv": wkv, "wq": wq2,
            "bq": bqc, "bk": bkc, "bv": bvc,
        })

    trace = bool(os.environ.get("ATTN_TRACE"))
    res = run_bass_kernel_spmd(nc, in_maps, core_ids=list(range(8)),
                               trace=trace)
    if trace:
        LAST_EXEC_TIME_NS = res.exec_time_ns
    return np.stack([res.results[i]["out"] for i in range(8)]).astype(
        np.float32)
